# revision 19
# baseline (speedup 1.0000x reference)
"""Distributed Trainium2 kernel for a pre-norm transformer block.

Sharding: 4096 tokens split 512/core over 8 cores; cores 0-3 hold batch 0,
cores 4-7 batch 1. k/v are AllGathered (bf16) within each 4-core batch group.
Matmuls run in bf16 (full PE rate) with fp32 PSUM accumulation; the residual
stream, LN statistics and softmax stay fp32. Weights arrive pre-transposed
and pre-cast to bf16 from the host; LN gain/bias are folded into the weights.
"""

import numpy as np
import ml_dtypes

import concourse.bass as bass
import concourse.mybir as mybir
import concourse.tile as tile
from concourse import bacc
from concourse.bass_utils import run_bass_kernel_spmd
from concourse.masks import make_identity

F32 = mybir.dt.float32
BF16 = mybir.dt.bfloat16
AF = mybir.ActivationFunctionType

B, S, D, H, HD, FF = 2, 2048, 1024, 16, 64, 4096
NC = 8
T = (B * S) // NC          # 512 tokens per core
GROUP = 4                  # cores per batch group
SK = S // 128              # 16 key tiles per batch
EPS = 1e-5
SCALE = 1.0 / np.sqrt(HD)


def build_graph():
    nc = bacc.Bacc("TRN2", target_bir_lowering=False, debug=False, num_devices=NC)

    x_ext = nc.declare_dram_parameter("x", [T, D], F32, False)
    wq_ext = nc.declare_dram_parameter("wq", [D, D], BF16, False)
    wk_ext = nc.declare_dram_parameter("wk", [D, D], BF16, False)
    wv_ext = nc.declare_dram_parameter("wv", [D, D], BF16, False)
    bq_ext = nc.declare_dram_parameter("bq", [D], F32, False)
    bk_ext = nc.declare_dram_parameter("bk", [D], F32, False)
    bv_ext = nc.declare_dram_parameter("bv", [D], F32, False)
    wo_ext = nc.declare_dram_parameter("wo", [D, D], BF16, False)
    bo_ext = nc.declare_dram_parameter("bo", [D], F32, False)
    w1_ext = nc.declare_dram_parameter("w1", [D, FF], BF16, False)
    b1_ext = nc.declare_dram_parameter("b1", [FF], F32, False)
    w2_ext = nc.declare_dram_parameter("w2", [FF, D], BF16, False)
    b2_ext = nc.declare_dram_parameter("b2", [D], F32, False)
    out_ext = nc.declare_dram_parameter("out", [T, D], F32, True)

    rg = [[0, 1, 2, 3], [4, 5, 6, 7]]

    with tile.TileContext(nc) as tc:
        with (
            tc.tile_pool(name="dram", bufs=1, space="DRAM") as dram,
            tc.tile_pool(name="const", bufs=1) as const,
            tc.tile_pool(name="stats", bufs=4) as stats,
            tc.tile_pool(name="xin", bufs=3) as xin,
            tc.tile_pool(name="big", bufs=1) as big,
            tc.tile_pool(name="wp", bufs=3) as wp,
            tc.tile_pool(name="wbig", bufs=2) as wbig,
            tc.tile_pool(name="kv", bufs=2) as kvp,
            tc.tile_pool(name="pt", bufs=4) as ptp,
            tc.tile_pool(name="small", bufs=2) as small,
            tc.tile_pool(name="stage", bufs=3) as stg,
            tc.tile_pool(name="at", bufs=2) as atp,
            tc.tile_pool(name="ps", bufs=2, space="PSUM") as psA,
            tc.tile_pool(name="ps_sc", bufs=1, space="PSUM") as psS,
            tc.tile_pool(name="ps_av", bufs=1, space="PSUM") as psV,
            tc.tile_pool(name="ps_tp", bufs=1, space="PSUM") as psT,
        ):
            # ---- DRAM bounce buffers for the k/v AllGathers (bf16) ----
            k_in = dram.tile([D, T], BF16)
            k_all = dram.tile([GROUP * D, T], BF16)
            v_in = dram.tile([T, D], BF16)
            v_all = dram.tile([GROUP * T, D], BF16)

            # ---- constants ----
            ident_b = const.tile([128, 128], BF16)
            make_identity(nc, ident_b[:])
            ident_f = const.tile([128, 128], F32)
            make_identity(nc, ident_f[:])
            ones_row = const.tile([1, 64], BF16)
            nc.any.memset(ones_row[:], 1.0)
            zero_c = const.tile([128, 1], F32)
            nc.any.memset(zero_c[:], 0.0)
            eps_c = const.tile([128, 1], F32)
            nc.any.memset(eps_c[:], EPS)

            bq_sb = const.tile([64, H], F32)
            nc.sync.dma_start(bq_sb[:], bq_ext.ap().rearrange("(h d) -> d h", d=64))
            bk_sb = const.tile([128, 8], F32)
            nc.sync.dma_start(bk_sb[:], bk_ext.ap().rearrange("(o p) -> p o", p=128))
            bv_sb = const.tile([128, 8], F32)
            nc.sync.dma_start(bv_sb[:], bv_ext.ap().rearrange("(o p) -> p o", p=128))
            bo_sb = const.tile([128, 8], F32)
            nc.sync.dma_start(bo_sb[:], bo_ext.ap().rearrange("(o p) -> p o", p=128))
            b1_sb = const.tile([128, 32], F32)
            nc.sync.dma_start(b1_sb[:], b1_ext.ap().rearrange("(o p) -> p o", p=128))
            b2_sb = const.tile([128, 8], F32)
            nc.sync.dma_start(b2_sb[:], b2_ext.ap().rearrange("(o p) -> p o", p=128))

            # ---- long-lived activations ----
            x2_sb = big.tile([128, 4, D], F32, name="x2")            # 16KB/p
            hT = big.tile([128, 8, T], BF16, name="hT", tag="resid")  # 8KB/p
            valsT = big.tile([128, 8, T], BF16, name="valsT", tag="vh2")
            qT = big.tile([64, H, T], BF16, name="qT")               # 16KB/p

            def layernorm(src_fn, dst_T):
                """src_fn(t) -> token-major fp32 [128, D] AP. Writes the
                transposed bf16 normalized output into dst_T [128, 8, T]."""
                for t in range(4):
                    xt = src_fn(t)
                    sm = stats.tile([128, 1], F32, tag="sm")
                    sq = stats.tile([128, 1], F32, tag="sq")
                    nc.vector.tensor_reduce(sm[:], xt, mybir.AxisListType.X,
                                            mybir.AluOpType.add)
                    scr = xin.tile([128, D], F32, tag="sqscr", bufs=2)
                    nc.scalar.activation(scr[:], xt, AF.Square, bias=zero_c[:],
                                         accum_out=sq[:])
                    m = stats.tile([128, 1], F32, tag="m")
                    nc.vector.tensor_scalar_mul(m[:], sm[:], 1.0 / D)
                    msq = stats.tile([128, 1], F32, tag="msq")
                    nc.vector.tensor_tensor(msq[:], m[:], m[:], mybir.AluOpType.mult)
                    var = stats.tile([128, 1], F32, tag="var")
                    nc.vector.tensor_scalar(var[:], sq[:], 1.0 / D, None,
                                            mybir.AluOpType.mult)
                    nc.vector.tensor_tensor(var[:], var[:], msq[:],
                                            mybir.AluOpType.subtract)
                    std = stats.tile([128, 1], F32, tag="std")
                    nc.scalar.activation(std[:], var[:], AF.Sqrt, bias=eps_c[:])
                    rs = stats.tile([128, 1], F32, tag="rs")
                    nc.vector.reciprocal(rs[:], std[:])
                    ht = xin.tile([128, D], BF16, tag="hnorm", bufs=2)
                    nc.vector.tensor_scalar(ht[:], xt, m[:], rs[:],
                                            mybir.AluOpType.subtract,
                                            mybir.AluOpType.mult)
                    for j in range(8):
                        pt = psT.tile([128, 128], BF16, tag="tp")
                        nc.tensor.transpose(pt[:], ht[:, j * 128:(j + 1) * 128],
                                            ident_b[:])
                        nc.vector.tensor_copy(out=dst_T[:, j, t * 128:(t + 1) * 128],
                                              in_=pt[:])

            # ================= stage A: LN1 =================
            def src_x(t):
                xt = xin.tile([128, D], F32, tag="x", bufs=2)
                nc.sync.dma_start(xt[:], x_ext[t * 128:(t + 1) * 128, :])
                return xt[:]
            layernorm(src_x, hT)

            # ============ stage B: K,V projections ============
            wk_ap = wk_ext.ap().rearrange("(o p) f -> p o f", p=128)
            wv_ap = wv_ext.ap().rearrange("(o p) f -> p o f", p=128)
            k_in_ap = k_in.rearrange("(o p) t -> p o t", p=128)
            for f in range(8):
                wt = wp.tile([128, 8, 128], BF16, tag="w")
                nc.sync.dma_start(wt[:], wk_ap[:, :, f * 128:(f + 1) * 128])
                ps = psA.tile([128, T], F32, tag="proj")
                for k in range(8):
                    nc.tensor.matmul(ps[:], wt[:, k, :], hT[:, k, :],
                                     start=(k == 0), stop=(k == 7))
                st = stg.tile([128, T], BF16, tag="kvstage")
                nc.vector.tensor_scalar_add(st[:], ps[:], bk_sb[:, f:f + 1])
                nc.sync.dma_start(k_in_ap[:, f, :], st[:])
            for f in range(8):
                wt = wp.tile([128, 8, 128], BF16, tag="w")
                nc.sync.dma_start(wt[:], wv_ap[:, :, f * 128:(f + 1) * 128])
                ps = psA.tile([128, T], F32, tag="proj")
                for k in range(8):
                    nc.tensor.matmul(ps[:], wt[:, k, :], hT[:, k, :],
                                     start=(k == 0), stop=(k == 7))
                st = stg.tile([128, T], BF16, tag="kvstage")
                nc.vector.tensor_scalar_add(st[:], ps[:], bv_sb[:, f:f + 1])
                # transpose v to token-major, straight to the DRAM bounce
                for t in range(4):
                    pt = psT.tile([128, 128], BF16, tag="tp")
                    nc.tensor.transpose(pt[:], st[:, t * 128:(t + 1) * 128],
                                        ident_b[:])
                    vst = stg.tile([128, 128], BF16, tag="vstage")
                    nc.vector.tensor_copy(out=vst[:], in_=pt[:])
                    nc.sync.dma_start(
                        v_in[t * 128:(t + 1) * 128, f * 128:(f + 1) * 128],
                        vst[:])

            # ============ stage C: AllGather k and v ============
            nc.gpsimd.collective_compute(
                "AllGather", mybir.AluOpType.bypass, replica_groups=rg,
                ins=[k_in[:].opt()], outs=[k_all[:].opt()])
            nc.gpsimd.collective_compute(
                "AllGather", mybir.AluOpType.bypass, replica_groups=rg,
                ins=[v_in[:].opt()], outs=[v_all[:].opt()])

            # ======== stage D0: Q projection for all heads (overlaps AG) ====
            wq_ap = wq_ext.ap().rearrange("(o p) f -> p o f", p=128)
            for fq in range(4):       # 4 chunks of 256 q-features = 4 heads
                wt = wp.tile([128, 8, 256], BF16, tag="wq")
                nc.sync.dma_start(wt[:], wq_ap[:, :, fq * 256:(fq + 1) * 256])
                for hh in range(4):   # head within chunk
                    h = fq * 4 + hh
                    qps = psV.tile([128, T], F32, tag="qproj")
                    for k in range(8):
                        nc.tensor.matmul(qps[:64, :],
                                         wt[:, k, hh * 64:(hh + 1) * 64],
                                         hT[:, k, :],
                                         start=(k == 0), stop=(k == 7))
                    nc.vector.tensor_scalar_add(qT[:, h, :], qps[:64, :],
                                                bq_sb[:, h:h + 1])

            # ============ stage D: attention per head ============
            k_all_ap = k_all.rearrange("(c o p) t -> p c o t", c=GROUP, p=128)
            v_all_ap = v_all.rearrange("(n p) d -> p n d", p=128)
            for h in range(H):
                kh = kvp.tile([64, GROUP, T], BF16, tag="kh")
                nc.sync.dma_start(
                    kh[:], k_all_ap
                    .rearrange("p c o t -> c o p t")
                    [:, h // 2, (h % 2) * 64:(h % 2) * 64 + 64, :]
                    .rearrange("c p t -> p c t"))
                vh = kvp.tile([128, SK, 65], BF16, tag="vh")
                nc.sync.dma_start(vh[:, :, :64],
                                  v_all_ap[:, :, h * 64:(h + 1) * 64])
                nc.any.memset(vh[:, :, 64:65], 1.0)

                avp = psV.tile([128, T], F32, tag="av")
                for tk in range(SK):
                    c, j = tk // 4, tk % 4
                    sps = psS.tile([128, T], F32, tag="sc", bufs=2)
                    nc.tensor.matmul(sps[:],
                                     kh[:, c, j * 128:(j + 1) * 128],
                                     qT[:, h, :], start=True, stop=True)
                    pt = ptp.tile([128, T], BF16, tag="pt")
                    nc.scalar.activation(pt[:], sps[:], AF.Exp, bias=zero_c[:],
                                         scale=SCALE)
                    nc.tensor.matmul(avp[:65, :], vh[:, tk, :], pt[:],
                                     start=(tk == 0), stop=(tk == SK - 1))

                # normalize: vals /= l  (row 64 of avp holds l)
                rlf = small.tile([1, T], F32, tag="rlf")
                nc.vector.reciprocal(rlf[:], avp[64:65, :])
                rl = small.tile([1, T], BF16, tag="rl")
                nc.vector.tensor_copy(out=rl[:], in_=rlf[:])
                rlp = psS.tile([128, T], F32, tag="rlrep")
                nc.tensor.matmul(rlp[:64, :], ones_row[:], rl[:],
                                 start=True, stop=True)
                rls = small.tile([64, T], F32, tag="rls")
                nc.vector.tensor_copy(out=rls[:], in_=rlp[:64, :])
                nc.vector.tensor_tensor(
                    valsT[(h % 2) * 64:(h % 2) * 64 + 64, h // 2, :],
                    avp[:64, :], rls[:], mybir.AluOpType.mult)

            # ============ stage E: o-projection + residual ============
            wo_ap = wo_ext.ap().rearrange("(o p) f -> p o f", p=128)
            xr_tiles = []
            for t in range(4):
                xr = xin.tile([128, D], F32, tag="xr", bufs=4)
                nc.sync.dma_start(xr[:], x_ext[t * 128:(t + 1) * 128, :])
                xr_tiles.append(xr)
            for f in range(8):
                wt = wp.tile([128, 8, 128], BF16, tag="w")
                nc.sync.dma_start(wt[:], wo_ap[:, :, f * 128:(f + 1) * 128])
                ps = psA.tile([128, T], F32, tag="proj")
                for k in range(8):
                    nc.tensor.matmul(ps[:], wt[:, k, :], valsT[:, k, :],
                                     start=(k == 0), stop=(k == 7))
                st = stg.tile([128, T], F32, tag="ostage")
                nc.vector.tensor_scalar_add(st[:], ps[:], bo_sb[:, f:f + 1])
                for t in range(4):
                    pt = psT.tile([128, 128], F32, tag="tp")
                    nc.tensor.transpose(pt[:], st[:, t * 128:(t + 1) * 128],
                                        ident_f[:])
                    nc.vector.tensor_tensor(
                        x2_sb[:, t, f * 128:(f + 1) * 128], pt[:],
                        xr_tiles[t][:, f * 128:(f + 1) * 128],
                        mybir.AluOpType.add)

            # ============ stage F: LN2 ============
            h2T = big.tile([128, 8, T], BF16, name="h2T", tag="vh2")
            layernorm(lambda t: x2_sb[:, t, :], h2T)

            # ============ stage G: MLP ============
            y2T = big.tile([128, 8, T], F32, name="y2T", tag="resid2")
            w1_ap = w1_ext.ap().rearrange("(o p) f -> p o f", p=128)
            w2_ap = w2_ext.ap().rearrange("(o p) f -> p o f", p=128)
            for quarter in range(4):
                aT = atp.tile([128, 8, T], BF16, tag="aT")
                for fi in range(2):
                    wt = wbig.tile([128, 8, T], BF16, tag="wbig")
                    f0 = quarter * 1024 + fi * 512
                    nc.sync.dma_start(wt[:], w1_ap[:, :, f0:f0 + 512])
                    for ff in range(4):
                        f = fi * 4 + ff          # f-tile within quarter
                        fg = quarter * 8 + f     # global f-tile (of 32)
                        ps = psA.tile([128, T], F32, tag="proj")
                        for k in range(8):
                            nc.tensor.matmul(
                                ps[:], wt[:, k, ff * 128:(ff + 1) * 128],
                                h2T[:, k, :],
                                start=(k == 0), stop=(k == 7))
                        nc.scalar.activation(aT[:, f, :], ps[:], AF.Gelu,
                                             bias=b1_sb[:, fg:fg + 1])
                # MLP2 partial pass over this quarter's aT
                for f in range(8):
                    wt = wp.tile([128, 8, 128], BF16, tag="w")
                    nc.sync.dma_start(
                        wt[:], w2_ap[:, quarter * 8:quarter * 8 + 8,
                                     f * 128:(f + 1) * 128])
                    ps = psA.tile([128, T], F32, tag="proj")
                    for k in range(8):
                        nc.tensor.matmul(ps[:], wt[:, k, :], aT[:, k, :],
                                         start=(k == 0), stop=(k == 7))
                    if quarter == 0:
                        nc.vector.tensor_scalar_add(y2T[:, f, :], ps[:],
                                                    b2_sb[:, f:f + 1])
                    else:
                        nc.vector.tensor_tensor(y2T[:, f, :], ps[:],
                                                y2T[:, f, :],
                                                mybir.AluOpType.add)

            # final: transpose y2, add residual, write out
            for f in range(8):
                for t in range(4):
                    pt = psT.tile([128, 128], F32, tag="tp")
                    nc.tensor.transpose(pt[:], y2T[:, f, t * 128:(t + 1) * 128],
                                        ident_f[:])
                    ost = stg.tile([128, 128], F32, tag="outstage")
                    nc.vector.tensor_tensor(ost[:], pt[:],
                                            x2_sb[:, t, f * 128:(f + 1) * 128],
                                            mybir.AluOpType.add)
                    nc.sync.dma_start(
                        out_ext[t * 128:(t + 1) * 128, f * 128:(f + 1) * 128],
                        ost[:])

    nc.compile()
    return nc


_CACHED = {}


def _prep_inputs(x, g1, b1, Wqkv, bqkv, Wo, bo, g2, b2, W1, b1m, W2, b2m):
    bf16 = ml_dtypes.bfloat16
    x = np.asarray(x, np.float32).reshape(B * S, D)
    Wqkv = np.asarray(Wqkv, np.float32)
    # fold LN1 gain/bias into qkv weights
    W_eff = Wqkv * np.asarray(g1, np.float32)[None, :]
    b_eff = np.asarray(bqkv, np.float32) + Wqkv @ np.asarray(b1, np.float32)
    idx = np.arange(H)[:, None] * 3 * HD + np.arange(HD)[None, :]
    q_rows = idx.ravel()
    k_rows = (idx + HD).ravel()
    v_rows = (idx + 2 * HD).ravel()
    WqT = np.ascontiguousarray(W_eff[q_rows].T).astype(bf16)
    WkT = np.ascontiguousarray(W_eff[k_rows].T).astype(bf16)
    WvT = np.ascontiguousarray(W_eff[v_rows].T).astype(bf16)
    bq, bk, bv = b_eff[q_rows].copy(), b_eff[k_rows].copy(), b_eff[v_rows].copy()
    WoT = np.ascontiguousarray(np.asarray(Wo, np.float32).T).astype(bf16)
    W1 = np.asarray(W1, np.float32)
    W1T = np.ascontiguousarray(
        (W1 * np.asarray(g2, np.float32)[None, :]).T).astype(bf16)
    b1m_eff = np.asarray(b1m, np.float32) + W1 @ np.asarray(b2, np.float32)
    W2T = np.ascontiguousarray(np.asarray(W2, np.float32).T).astype(bf16)
    shared = dict(wq=WqT, wk=WkT, wv=WvT, bq=bq, bk=bk, bv=bv,
                  wo=WoT, bo=np.asarray(bo, np.float32),
                  w1=W1T, b1=b1m_eff, w2=W2T, b2=np.asarray(b2m, np.float32))
    in_maps = []
    for c in range(NC):
        m = dict(shared)
        m["x"] = np.ascontiguousarray(x[c * T:(c + 1) * T])
        in_maps.append(m)
    return in_maps


def kernel(**inputs):
    if "nc" not in _CACHED:
        _CACHED["nc"] = build_graph()
    nc = _CACHED["nc"]
    in_maps = _prep_inputs(**inputs)
    res = run_bass_kernel_spmd(nc, in_maps, core_ids=list(range(NC)))
    outs = [res.results[c]["out"] for c in range(NC)]
    full = np.concatenate(outs, axis=0).reshape(B, S, D)
    return full.astype(np.float32)


# revision 21
# speedup vs baseline: 1.0099x; 1.0099x over previous
"""Distributed Trainium2 kernel for a pre-norm transformer block.

Sharding: 4096 tokens split 512/core over 8 cores; cores 0-3 hold batch 0,
cores 4-7 batch 1. k/v are AllGathered (bf16) within each 4-core batch group,
split into 4 per-head-group collectives so attention on head group g overlaps
the AllGathers of groups g+1..3. Matmuls run in bf16 (full PE rate) with fp32
PSUM accumulation; the residual stream, LN statistics and softmax stay fp32.
Weights arrive pre-transposed and pre-cast to bf16 from the host; LN
gain/bias are folded into the weights.
"""

import numpy as np
import ml_dtypes

import concourse.bass as bass
import concourse.mybir as mybir
import concourse.tile as tile
from concourse import bacc
from concourse.bass_utils import run_bass_kernel_spmd
from concourse.masks import make_identity

F32 = mybir.dt.float32
BF16 = mybir.dt.bfloat16
AF = mybir.ActivationFunctionType

B, S, D, H, HD, FF = 2, 2048, 1024, 16, 64, 4096
NC = 8
T = (B * S) // NC          # 512 tokens per core
GROUP = 4                  # cores per batch group
SK = S // 128              # 16 key tiles per batch
NS = 4                     # number of split kv AllGathers (4 heads each)
KELE = 256 * T             # k elements per split (256 feature rows)
EPS = 1e-5
SCALE = 1.0 / np.sqrt(HD)


def build_graph():
    nc = bacc.Bacc("TRN2", target_bir_lowering=False, debug=False, num_devices=NC)

    x_ext = nc.declare_dram_parameter("x", [T, D], F32, False)
    wq_ext = nc.declare_dram_parameter("wq", [D, D], BF16, False)
    wk_ext = nc.declare_dram_parameter("wk", [D, D], BF16, False)
    wv_ext = nc.declare_dram_parameter("wv", [D, D], BF16, False)
    bq_ext = nc.declare_dram_parameter("bq", [D], F32, False)
    bk_ext = nc.declare_dram_parameter("bk", [D], F32, False)
    bv_ext = nc.declare_dram_parameter("bv", [D], F32, False)
    wo_ext = nc.declare_dram_parameter("wo", [D, D], BF16, False)
    bo_ext = nc.declare_dram_parameter("bo", [D], F32, False)
    w1_ext = nc.declare_dram_parameter("w1", [D, FF], BF16, False)
    b1_ext = nc.declare_dram_parameter("b1", [FF], F32, False)
    w2_ext = nc.declare_dram_parameter("w2", [FF, D], BF16, False)
    b2_ext = nc.declare_dram_parameter("b2", [D], F32, False)
    out_ext = nc.declare_dram_parameter("out", [T, D], F32, True)

    rg = [[0, 1, 2, 3], [4, 5, 6, 7]]

    with tile.TileContext(nc) as tc:
        with (
            tc.tile_pool(name="dram", bufs=1, space="DRAM") as dram,
            tc.tile_pool(name="const", bufs=1) as const,
            tc.tile_pool(name="stats", bufs=4) as stats,
            tc.tile_pool(name="xin", bufs=3) as xin,
            tc.tile_pool(name="big", bufs=1) as big,
            tc.tile_pool(name="wp", bufs=3) as wp,
            tc.tile_pool(name="wbig", bufs=2) as wbig,
            tc.tile_pool(name="kv", bufs=2) as kvp,
            tc.tile_pool(name="pt", bufs=4) as ptp,
            tc.tile_pool(name="small", bufs=2) as small,
            tc.tile_pool(name="stage", bufs=3) as stg,
            tc.tile_pool(name="at", bufs=2) as atp,
        ):
            # ---- DRAM bounce buffers: one fused k+v AllGather per
            # 4-head group g.  kv_in[g,0] = k features [256, T];
            # kv_in[g,1] = v token-major [T, 256] (cols g*256..). ----
            kv_in = dram.tile([NS, 2, KELE], BF16)
            kv_all = dram.tile([NS, GROUP, 2, KELE], BF16)

            # ---- constants ----
            ident_b = const.tile([128, 128], BF16)
            make_identity(nc, ident_b[:])
            ident_f = const.tile([128, 128], F32)
            make_identity(nc, ident_f[:])
            ones_row = const.tile([1, 64], BF16)
            nc.any.memset(ones_row[:], 1.0)
            zero_c = const.tile([128, 1], F32)
            nc.any.memset(zero_c[:], 0.0)
            eps_c = const.tile([128, 1], F32)
            nc.any.memset(eps_c[:], EPS)

            bq_sb = const.tile([64, H], F32)
            nc.sync.dma_start(bq_sb[:], bq_ext.ap().rearrange("(h d) -> d h", d=64))
            bk_sb = const.tile([128, 8], F32)
            nc.sync.dma_start(bk_sb[:], bk_ext.ap().rearrange("(o p) -> p o", p=128))
            bv_sb = const.tile([128, 8], F32)
            nc.sync.dma_start(bv_sb[:], bv_ext.ap().rearrange("(o p) -> p o", p=128))
            bo_sb = const.tile([128, 8], F32)
            nc.sync.dma_start(bo_sb[:], bo_ext.ap().rearrange("(o p) -> p o", p=128))
            b1_sb = const.tile([128, 32], F32)
            nc.sync.dma_start(b1_sb[:], b1_ext.ap().rearrange("(o p) -> p o", p=128))
            b2_sb = const.tile([128, 8], F32)
            nc.sync.dma_start(b2_sb[:], b2_ext.ap().rearrange("(o p) -> p o", p=128))

            # ---- long-lived activations ----
            x2_sb = big.tile([128, 4, D], F32, name="x2")            # 16KB/p
            hT = big.tile([128, 8, T], BF16, name="hT", tag="resid")  # 8KB/p
            valsT = big.tile([128, 8, T], BF16, name="valsT", tag="vh2")
            qT = big.tile([64, H, T], BF16, name="qT")               # 16KB/p
            # static double-buffered v tiles with the softmax-sum ones
            # column pre-set (avoids a per-head memset)
            vh_ab = [big.tile([128, SK, 65], BF16, name=f"vh{i}")
                     for i in range(2)]
            for vv in vh_ab:
                nc.any.memset(vv[:, :, 64:65], 1.0)

            def layernorm(src_fn, dst_T, psT):
                """src_fn(t) -> token-major fp32 [128, D] AP. Writes the
                transposed bf16 normalized output into dst_T [128, 8, T]."""
                for t in range(4):
                    xt = src_fn(t)
                    sm = stats.tile([128, 1], F32, tag="sm")
                    sq = stats.tile([128, 1], F32, tag="sq")
                    nc.vector.tensor_reduce(sm[:], xt, mybir.AxisListType.X,
                                            mybir.AluOpType.add)
                    scr = xin.tile([128, D], F32, tag="sqscr", bufs=2)
                    nc.scalar.activation(scr[:], xt, AF.Square, bias=zero_c[:],
                                         accum_out=sq[:])
                    m = stats.tile([128, 1], F32, tag="m")
                    nc.vector.tensor_scalar_mul(m[:], sm[:], 1.0 / D)
                    msq = stats.tile([128, 1], F32, tag="msq")
                    nc.vector.tensor_tensor(msq[:], m[:], m[:], mybir.AluOpType.mult)
                    var = stats.tile([128, 1], F32, tag="var")
                    nc.vector.tensor_scalar(var[:], sq[:], 1.0 / D, None,
                                            mybir.AluOpType.mult)
                    nc.vector.tensor_tensor(var[:], var[:], msq[:],
                                            mybir.AluOpType.subtract)
                    std = stats.tile([128, 1], F32, tag="std")
                    nc.scalar.activation(std[:], var[:], AF.Sqrt, bias=eps_c[:])
                    rs = stats.tile([128, 1], F32, tag="rs")
                    nc.vector.reciprocal(rs[:], std[:])
                    ht = xin.tile([128, D], BF16, tag="hnorm", bufs=2)
                    nc.vector.tensor_scalar(ht[:], xt, m[:], rs[:],
                                            mybir.AluOpType.subtract,
                                            mybir.AluOpType.mult)
                    for j in range(8):
                        pt = psT.tile([128, 128], BF16, tag="tp")
                        nc.tensor.transpose(pt[:], ht[:, j * 128:(j + 1) * 128],
                                            ident_b[:])
                        nc.vector.tensor_copy(out=dst_T[:, j, t * 128:(t + 1) * 128],
                                              in_=pt[:])

            wk_ap = wk_ext.ap().rearrange("(o p) f -> p o f", p=128)
            wv_ap = wv_ext.ap().rearrange("(o p) f -> p o f", p=128)
            wq_ap = wq_ext.ap().rearrange("(o p) f -> p o f", p=128)

            with (
                tc.tile_pool(name="ps1_tp", bufs=1, space="PSUM") as psT1,
                tc.tile_pool(name="ps1_pr", bufs=2, space="PSUM") as psA1,
                tc.tile_pool(name="ps1_q", bufs=2, space="PSUM") as psQ1,
            ):
                # ================= stage A: LN1 =================
                def src_x(t):
                    xt = xin.tile([128, D], F32, tag="x", bufs=2)
                    nc.sync.dma_start(xt[:], x_ext[t * 128:(t + 1) * 128, :])
                    return xt[:]
                layernorm(src_x, hT, psT1)

                # ===== stage B/C: K,V projections + split AllGathers =====
                for g in range(NS):
                    k_view = kv_in[g, 0, :].rearrange("(r t) -> r t", r=256)
                    v_view = kv_in[g, 1, :].rearrange("(t c) -> t c", t=T)
                    for fi in range(2):
                        f = g * 2 + fi
                        wt = wp.tile([128, 8, 128], BF16, tag="w")
                        nc.sync.dma_start(wt[:], wk_ap[:, :, f * 128:(f + 1) * 128])
                        ps = psA1.tile([128, T], F32, tag="proj")
                        for k in range(8):
                            nc.tensor.matmul(ps[:], wt[:, k, :], hT[:, k, :],
                                             start=(k == 0), stop=(k == 7))
                        st = stg.tile([128, T], BF16, tag="kvstage")
                        nc.vector.tensor_scalar_add(st[:], ps[:], bk_sb[:, f:f + 1])
                        nc.sync.dma_start(k_view[fi * 128:(fi + 1) * 128, :], st[:])
                    for fi in range(2):
                        f = g * 2 + fi
                        wt = wp.tile([128, 8, 128], BF16, tag="w")
                        nc.sync.dma_start(wt[:], wv_ap[:, :, f * 128:(f + 1) * 128])
                        ps = psA1.tile([128, T], F32, tag="proj")
                        for k in range(8):
                            nc.tensor.matmul(ps[:], wt[:, k, :], hT[:, k, :],
                                             start=(k == 0), stop=(k == 7))
                        st = stg.tile([128, T], BF16, tag="kvstage")
                        nc.vector.tensor_scalar_add(st[:], ps[:], bv_sb[:, f:f + 1])
                        for t in range(4):
                            pt = psT1.tile([128, 128], BF16, tag="tp")
                            nc.tensor.transpose(pt[:], st[:, t * 128:(t + 1) * 128],
                                                ident_b[:])
                            vst = stg.tile([128, 128], BF16, tag="vstage")
                            nc.vector.tensor_copy(out=vst[:], in_=pt[:])
                            nc.sync.dma_start(
                                v_view[t * 128:(t + 1) * 128,
                                       fi * 128:(fi + 1) * 128], vst[:])
                    nc.gpsimd.collective_compute(
                        "AllGather", mybir.AluOpType.bypass, replica_groups=rg,
                        ins=[kv_in[g, :, :].opt()], outs=[kv_all[g, :, :, :].opt()])

                # ======== stage D0: Q projection for all heads ========
                for fq in range(4):
                    wt = wp.tile([128, 8, 256], BF16, tag="wq")
                    nc.sync.dma_start(wt[:], wq_ap[:, :, fq * 256:(fq + 1) * 256])
                    for hh in range(4):
                        h = fq * 4 + hh
                        qps = psQ1.tile([128, T], F32, tag="qproj")
                        for k in range(8):
                            nc.tensor.matmul(qps[:64, :],
                                             wt[:, k, hh * 64:(hh + 1) * 64],
                                             hT[:, k, :],
                                             start=(k == 0), stop=(k == 7))
                        nc.vector.tensor_scalar_add(qT[:, h, :], qps[:64, :],
                                                    bq_sb[:, h:h + 1])

            # ============ stage D: attention per head ============
            with (
                tc.tile_pool(name="ps2_sc", bufs=3, space="PSUM") as psS,
                tc.tile_pool(name="ps2_av", bufs=2, space="PSUM") as psV,
                tc.tile_pool(name="ps2_rl", bufs=1, space="PSUM") as psR,
            ):
                for h in range(H):
                    g, hh = h // 4, h % 4
                    kh = kvp.tile([64, GROUP, T], BF16, tag="kh")
                    nc.sync.dma_start(
                        kh[:], kv_all[g, :, 0, :]
                        .rearrange("c (r t) -> c r t", r=256)
                        [:, hh * 64:(hh + 1) * 64, :]
                        .rearrange("c r t -> r c t"))
                    vh = vh_ab[h % 2]
                    for c in range(GROUP):
                        nc.sync.dma_start(
                            vh[:, c * 4:(c + 1) * 4, :64],
                            kv_all[g, c, 1, :]
                            .rearrange("(n p cc) -> p n cc", n=4, p=128)
                            [:, :, hh * 64:(hh + 1) * 64])

                    avp = psV.tile([128, T], F32, tag="av")
                    for tk in range(SK):
                        c, j = tk // 4, tk % 4
                        sps = psS.tile([128, T], F32, tag="sc")
                        nc.tensor.matmul(sps[:],
                                         kh[:, c, j * 128:(j + 1) * 128],
                                         qT[:, h, :], start=True, stop=True)
                        pt = ptp.tile([128, T], BF16, tag="pt")
                        nc.scalar.activation(pt[:], sps[:], AF.Exp,
                                             bias=zero_c[:], scale=SCALE)
                        nc.tensor.matmul(avp[:65, :], vh[:, tk, :], pt[:],
                                         start=(tk == 0), stop=(tk == SK - 1))

                    # normalize: vals /= l  (row 64 of avp holds l)
                    rlf = small.tile([1, T], F32, tag="rlf")
                    nc.vector.reciprocal(rlf[:], avp[64:65, :])
                    rl = small.tile([1, T], BF16, tag="rl")
                    nc.vector.tensor_copy(out=rl[:], in_=rlf[:])
                    rlp = psR.tile([128, T], F32, tag="rlrep")
                    nc.tensor.matmul(rlp[:64, :], ones_row[:], rl[:],
                                     start=True, stop=True)
                    rls = small.tile([64, T], F32, tag="rls")
                    nc.vector.tensor_copy(out=rls[:], in_=rlp[:64, :])
                    nc.vector.tensor_tensor(
                        valsT[(h % 2) * 64:(h % 2) * 64 + 64, h // 2, :],
                        avp[:64, :], rls[:], mybir.AluOpType.mult)

            # ============ stages E/F/G ============
            with (
                tc.tile_pool(name="ps3_tp", bufs=2, space="PSUM") as psT3,
                tc.tile_pool(name="ps3_pr", bufs=2, space="PSUM") as psA3,
            ):
                # ---- stage E: o-projection + residual ----
                wo_ap = wo_ext.ap().rearrange("(o p) f -> p o f", p=128)
                xr_tiles = []
                for t in range(4):
                    xr = xin.tile([128, D], F32, tag="xr", bufs=4)
                    nc.sync.dma_start(xr[:], x_ext[t * 128:(t + 1) * 128, :])
                    xr_tiles.append(xr)
                for f in range(8):
                    wt = wp.tile([128, 8, 128], BF16, tag="w")
                    nc.sync.dma_start(wt[:], wo_ap[:, :, f * 128:(f + 1) * 128])
                    ps = psA3.tile([128, T], F32, tag="proj")
                    for k in range(8):
                        nc.tensor.matmul(ps[:], wt[:, k, :], valsT[:, k, :],
                                         start=(k == 0), stop=(k == 7))
                    st = stg.tile([128, T], F32, tag="ostage")
                    nc.vector.tensor_scalar_add(st[:], ps[:], bo_sb[:, f:f + 1])
                    for t in range(4):
                        pt = psT3.tile([128, 128], F32, tag="tp")
                        nc.tensor.transpose(pt[:], st[:, t * 128:(t + 1) * 128],
                                            ident_f[:])
                        nc.vector.tensor_tensor(
                            x2_sb[:, t, f * 128:(f + 1) * 128], pt[:],
                            xr_tiles[t][:, f * 128:(f + 1) * 128],
                            mybir.AluOpType.add)

                # ---- stage F: LN2 ----
                h2T = big.tile([128, 8, T], BF16, name="h2T", tag="vh2")
                layernorm(lambda t: x2_sb[:, t, :], h2T, psT3)

                # ---- stage G: MLP ----
                y2T = big.tile([128, 8, T], F32, name="y2T", tag="resid2")
                w1_ap = w1_ext.ap().rearrange("(o p) f -> p o f", p=128)
                w2_ap = w2_ext.ap().rearrange("(o p) f -> p o f", p=128)
                for quarter in range(4):
                    aT = atp.tile([128, 8, T], BF16, tag="aT")
                    for fi in range(4):
                        wt = wbig.tile([128, 8, 256], BF16, tag="wbig")
                        f0 = quarter * 1024 + fi * 256
                        nc.sync.dma_start(wt[:], w1_ap[:, :, f0:f0 + 256])
                        for ff in range(2):
                            f = fi * 2 + ff          # f-tile within quarter
                            fg = quarter * 8 + f     # global f-tile (of 32)
                            ps = psA3.tile([128, T], F32, tag="proj")
                            for k in range(8):
                                nc.tensor.matmul(
                                    ps[:], wt[:, k, ff * 128:(ff + 1) * 128],
                                    h2T[:, k, :],
                                    start=(k == 0), stop=(k == 7))
                            nc.scalar.activation(aT[:, f, :], ps[:], AF.Gelu,
                                                 bias=b1_sb[:, fg:fg + 1])
                    # MLP2 partial pass over this quarter's aT
                    for f in range(8):
                        wt = wp.tile([128, 8, 128], BF16, tag="w")
                        nc.sync.dma_start(
                            wt[:], w2_ap[:, quarter * 8:quarter * 8 + 8,
                                         f * 128:(f + 1) * 128])
                        ps = psA3.tile([128, T], F32, tag="proj")
                        for k in range(8):
                            nc.tensor.matmul(ps[:], wt[:, k, :], aT[:, k, :],
                                             start=(k == 0), stop=(k == 7))
                        if quarter == 0:
                            nc.vector.tensor_scalar_add(y2T[:, f, :], ps[:],
                                                        b2_sb[:, f:f + 1])
                        else:
                            nc.vector.tensor_tensor(y2T[:, f, :], ps[:],
                                                    y2T[:, f, :],
                                                    mybir.AluOpType.add)

                # final: transpose y2, add residual, write out
                for f in range(8):
                    for t in range(4):
                        pt = psT3.tile([128, 128], F32, tag="tp")
                        nc.tensor.transpose(pt[:],
                                            y2T[:, f, t * 128:(t + 1) * 128],
                                            ident_f[:])
                        ost = stg.tile([128, 128], F32, tag="outstage")
                        nc.vector.tensor_tensor(ost[:], pt[:],
                                                x2_sb[:, t, f * 128:(f + 1) * 128],
                                                mybir.AluOpType.add)
                        nc.sync.dma_start(
                            out_ext[t * 128:(t + 1) * 128, f * 128:(f + 1) * 128],
                            ost[:])

    nc.compile()
    return nc


_CACHED = {}


def _prep_inputs(x, g1, b1, Wqkv, bqkv, Wo, bo, g2, b2, W1, b1m, W2, b2m):
    bf16 = ml_dtypes.bfloat16
    x = np.asarray(x, np.float32).reshape(B * S, D)
    Wqkv = np.asarray(Wqkv, np.float32)
    # fold LN1 gain/bias into qkv weights
    W_eff = Wqkv * np.asarray(g1, np.float32)[None, :]
    b_eff = np.asarray(bqkv, np.float32) + Wqkv @ np.asarray(b1, np.float32)
    idx = np.arange(H)[:, None] * 3 * HD + np.arange(HD)[None, :]
    q_rows = idx.ravel()
    k_rows = (idx + HD).ravel()
    v_rows = (idx + 2 * HD).ravel()
    WqT = np.ascontiguousarray(W_eff[q_rows].T).astype(bf16)
    WkT = np.ascontiguousarray(W_eff[k_rows].T).astype(bf16)
    WvT = np.ascontiguousarray(W_eff[v_rows].T).astype(bf16)
    bq, bk, bv = b_eff[q_rows].copy(), b_eff[k_rows].copy(), b_eff[v_rows].copy()
    WoT = np.ascontiguousarray(np.asarray(Wo, np.float32).T).astype(bf16)
    W1 = np.asarray(W1, np.float32)
    W1T = np.ascontiguousarray(
        (W1 * np.asarray(g2, np.float32)[None, :]).T).astype(bf16)
    b1m_eff = np.asarray(b1m, np.float32) + W1 @ np.asarray(b2, np.float32)
    W2T = np.ascontiguousarray(np.asarray(W2, np.float32).T).astype(bf16)
    shared = dict(wq=WqT, wk=WkT, wv=WvT, bq=bq, bk=bk, bv=bv,
                  wo=WoT, bo=np.asarray(bo, np.float32),
                  w1=W1T, b1=b1m_eff, w2=W2T, b2=np.asarray(b2m, np.float32))
    in_maps = []
    for c in range(NC):
        m = dict(shared)
        m["x"] = np.ascontiguousarray(x[c * T:(c + 1) * T])
        in_maps.append(m)
    return in_maps


def kernel(**inputs):
    if "nc" not in _CACHED:
        _CACHED["nc"] = build_graph()
    nc = _CACHED["nc"]
    in_maps = _prep_inputs(**inputs)
    res = run_bass_kernel_spmd(nc, in_maps, core_ids=list(range(NC)))
    outs = [res.results[c]["out"] for c in range(NC)]
    full = np.concatenate(outs, axis=0).reshape(B, S, D)
    return full.astype(np.float32)
